# revision 21
# baseline (speedup 1.0000x reference)
"""Conv2d(128->256, 3x3, pad=1) over a 256x256 image, sharded across 8 trn2 cores.

Strategy
--------
x: (C_in=128, H=256, W=256) f32, weight: (256, 128, 3, 3), bias: (256,1,1).
C_in == 128 maps exactly onto the SBUF partition (contraction) dim, so the
conv is 9 accumulated matmuls (one per kernel tap) per output tile:

    out[co, y, x] = sum_{ky,kx} W[ky,kx].T @ xpad[:, y+ky, x+kx]   + bias

Sharding: split H across the 8 cores (32 output rows each). Each core gets a
pre-padded slice xpad (128, 34, 258) with halo rows / zero borders prepared on
the host, so the device program is uniform SPMD. Per core, output tiles are
2 rows x 256 cols = 512 pixels (one fp32 PSUM bank); for each tile and each
128-channel output half we accumulate the taps, then VectorE applies
(psum * 1/512 + bias) on the PSUM->SBUF copy and the tile is DMA'd to DRAM
as fp16 (host upconverts; ~2^-11 rounding is far inside the error budget).

Mixed precision: 7 taps run as fp16 matmuls (full 16-bit PE rate, fp32 PSUM
accumulation). The last two taps (2,1),(2,2) run as ONE fp8e4m3 DoubleRow
matmul: the PE packs two fp8 weights per cell (virtual K=256), contracting
both taps in a single pass at ~1.1x the cost of one fp16 matmul instead of
two. Measured end-to-end rel err of this split on the reference distribution
is ~1.74e-2 (< the 2e-2 gate); pure fp8 would be ~3.4e-2 and fails.

All weights (fp16 and fp8) are pre-scaled by 512 on the host so the fp8
values land in e4m3's normal range (|w| <= 0.0295 would otherwise be ~50%
denormal, 20%+ quantization error); PSUM therefore holds 512*conv and the
bias copy fuses the 1/512 descale.

DMA: the two HWDGE queues (sync- and scalar-engine triggered) share the 16
DMA engines, so a concurrent bulk transfer steals wire bandwidth from the
critical path per-packet (and Tile's scheduler hoists dep-free dma_starts,
so a second queue cannot be held back). A single sync-engine queue therefore
carries ALL transfers in strict need-order. The fp8 (DoubleRow) matmuls of
tile t are deferred to after tile t+2's fp16 block so their data is never on
the critical path at all.
"""

import numpy as np
import ml_dtypes

import concourse.bass as bass
import concourse.tile as tile
from concourse import bacc, mybir
from concourse import bass_utils

N_CORES = 8
C_IN, C_OUT, KH, KW = 128, 256, 3, 3
H, W = 256, 256
H_S = H // N_CORES            # 32 output rows per core
HP, WP = H_S + 2, W + 2       # padded per-core input slice: 34 x 258
ROWS = 2                      # output rows per PSUM tile (N = ROWS*W = 512)
N_TILES = H_S // ROWS         # 16
N_HALF = C_OUT // 128         # 2 output-channel halves

F32 = mybir.dt.float32
F16 = mybir.dt.float16
F8 = mybir.dt.float8e4

# taps 0..6 = (0,0)..(2,0) in fp16; taps (2,1),(2,2) fused into one fp8
# DoubleRow matmul (see module docstring)
N_FP16_TAPS = 7
FP8_TAPS = [(2, 1), (2, 2)]
WSCALE = 512.0                # weight pre-scale; descaled in the bias copy

# x is split into row groups, each its own SBUF tile, so a group's matmuls
# can start as soon as its rows have landed (Tile deps are whole-tile). Each
# group covers GROUP_TILES[g] output tiles plus a 2-row halo overlap. The
# first group is 1 tile so the first matmuls start as early as possible.
# Few, large groups: every HWDGE trigger costs ~650ns of serial Sync-engine
# time and small transfers (small per-partition spans) get poor wire rates.
GROUP_TILES = [1, 3, 12]
assert sum(GROUP_TILES) == N_TILES
N_GROUPS = len(GROUP_TILES)

# fp8 pre-shifted x: [128, 2, H_S*W] per core, split into 2 groups (tiles).
X8_GROUP_TILES = [6, 10]
DR_DEFER = 2                  # run tile t's fp8 matmuls after tile t+2's fp16

# dep-free dummy matmuls issued at program start: they run while the input
# DMAs are in flight and lift the PE clock gate (HAM) out of its cold 1.2 GHz
# state before the real matmul stream begins. Sized to bridge engine-boot
# (~6.6us) to first-data (~10us): ~2.8us at the cold rate, with short
# N=128 warmups at the end for fine granularity. Keeping the PE busy until
# real data arrives also avoids an idle gap that would reset the HAM ramp.
WARMUP_512 = 6
WARMUP_128 = 2

# Set by test harness: TRACE=True makes the next kernel() call capture an
# NTFF profile; the BassKernelResults lands in LAST_RESULT.
TRACE = False
TRACE_KW = {}
LAST_RESULT = None

_NC_CACHE = {}


def _build():
    nc = bacc.Bacc(
        "TRN2",
        target_bir_lowering=False,
        debug=False,
        enable_asserts=False,
        num_devices=N_CORES,
    )
    x_d = nc.dram_tensor("x", [C_IN, HP, WP], F16, kind="ExternalInput").ap()
    x8_d = nc.dram_tensor("x8", [C_IN, 2, H_S * W], F8, kind="ExternalInput").ap()
    # fp16 taps 0..6 plus the fp32 bias folded in as 4 trailing fp16-typed
    # columns holding raw fp32 bytes (a separate bias tensor would be an
    # 8-byte-per-partition transfer — terrible packet size for the critical
    # DMA queue; and tensor_scalar requires an fp32 scalar2 operand)
    W_COLS = N_FP16_TAPS * C_OUT + 2 * N_HALF
    w_d = nc.dram_tensor("w", [C_IN, W_COLS], F16, kind="ExternalInput").ap()
    w8_d = nc.dram_tensor("w8", [C_IN, 2, C_OUT], F8, kind="ExternalInput").ap()
    # output laid out [p, half, y, x] (channel h*128+p at [p, h]) so ONE
    # DMA per unit moves both halves: each dma_start trigger costs ~600ns of
    # serial engine time, and the last units' triggers gate the kernel tail
    o_d = nc.dram_tensor(
        "out", [128, N_HALF, H_S, W], F16, kind="ExternalOutput"
    ).ap()

    with tile.TileContext(nc) as tc:
        with (
            tc.tile_pool(name="xin", bufs=1) as xpool,
            tc.tile_pool(name="x8in", bufs=1) as x8pool,
            tc.tile_pool(name="wts", bufs=1) as wpool,
            tc.tile_pool(name="acc", bufs=8, space="PSUM") as ppool,
            tc.tile_pool(name="outs", bufs=6) as opool,
        ):
            # PE warmup: dep-free. The scratch operand is a raw (statically
            # allocated) SBUF tensor that is never written — its garbage
            # contents stream through the PE and land in a scratch PSUM bank
            # nobody reads.
            warm_sb = nc.alloc_sbuf_tensor("warm_src", [128, ROWS * W], F16).ap()
            warm_ps = ppool.tile([128, ROWS * W], F32, tag="ps", name="ps")
            for _ in range(WARMUP_512):
                nc.tensor.matmul(warm_ps[:], warm_sb[:, :128], warm_sb[:])
            for _ in range(WARMUP_128):
                nc.tensor.matmul(warm_ps[:, :128], warm_sb[:, :128], warm_sb[:, :128])

            # three separate tiles: Tile dependencies are whole-tile, so
            # early taps must not wait for the later tap transfers
            wa_sb = wpool.tile([128, 3 * C_OUT], F16, tag="wa", name="wa")
            wb_sb = wpool.tile([128, 2 * C_OUT], F16, tag="wb", name="wb")
            wc_sb = wpool.tile([128, W_COLS - 5 * C_OUT], F16, tag="wc", name="wc")
            w8_sb = wpool.tile([128, 2, C_OUT], F8, tag="w8", name="w8")
            group_rows = [gt * ROWS + 2 for gt in GROUP_TILES]
            group_rows[0] = 3  # ky=2 rows live in the separate xg0b tile
            group_t0 = [sum(GROUP_TILES[:g]) for g in range(N_GROUPS)]
            x_groups = [
                xpool.tile([128, group_rows[g], WP], F16, tag=f"xg{g}", name=f"xg{g}")
                for g in range(N_GROUPS)
            ]
            xg0b = xpool.tile([128, 2, WP], F16, tag="xg0b", name="xg0b")
            x8_r0 = [0, X8_GROUP_TILES[0] * ROWS]
            x8_groups = [
                x8pool.tile(
                    [128, 2, gt * ROWS * W], F8, tag=f"x8g{g}", name=f"x8g{g}"
                )
                for g, gt in enumerate(X8_GROUP_TILES)
            ]

            # A single queue carries EVERYTHING in strict need-order: the
            # two HWDGE queues share the 16 DMA engines, so a "parallel"
            # second queue just steals wire bandwidth from the critical path
            # (and Tile's scheduler hoists dep-free dma_starts, so the
            # second queue cannot be held back). Need-order with the fp8
            # work deferred 2 tiles keeps every transfer ahead of its use.
            nc.sync.dma_start(x_groups[0][:], x_d[:, :3, :])
            nc.sync.dma_start(wa_sb[:], w_d[:, : 3 * C_OUT])
            nc.sync.dma_start(xg0b[:], x_d[:, 2:4, :])
            nc.sync.dma_start(wb_sb[:], w_d[:, 3 * C_OUT : 5 * C_OUT])
            nc.sync.dma_start(wc_sb[:], w_d[:, 5 * C_OUT :])
            r1 = group_t0[1] * ROWS
            nc.sync.dma_start(x_groups[1][:], x_d[:, r1 : r1 + group_rows[1], :])
            nc.sync.dma_start(w8_sb[:], w8_d[:])
            nc.sync.dma_start(
                x8_groups[0][:], x8_d[:, :, : X8_GROUP_TILES[0] * ROWS * W]
            )
            r2 = group_t0[2] * ROWS
            nc.sync.dma_start(x_groups[2][:], x_d[:, r2 : r2 + group_rows[2], :])
            nc.sync.dma_start(
                x8_groups[1][:], x8_d[:, :, X8_GROUP_TILES[0] * ROWS * W :]
            )

            def group_of_r0(r0):
                for g in reversed(range(N_GROUPS)):
                    if r0 >= group_t0[g] * ROWS:
                        return g
                raise AssertionError(r0)

            # processing units: 15 2-row tiles + two 1-row subtiles (the
            # split halves the bias-add + DMA latency off the final matmul)
            units = [(t * ROWS, ROWS) for t in range(N_TILES - 1)]
            units += [(H_S - 2, 1), (H_S - 1, 1)]
            live = {}

            def emit_fp16(u):
                r0, nrows = units[u]
                n = nrows * W
                g = group_of_r0(r0)
                yl = r0 - group_t0[g] * ROWS
                xg = x_groups[g]
                pss = [
                    ppool.tile([128, n], F32, tag="ps", name="ps")
                    for _ in range(N_HALF)
                ]
                live[u] = pss
                for k in range(N_FP16_TAPS):
                    ky, kx = divmod(k, KW)
                    if g == 0 and ky == 2:
                        rhs = xg0b[:, :nrows, kx : kx + W]
                    else:
                        rhs = xg[:, yl + ky : yl + ky + nrows, kx : kx + W]
                    if k < 3:
                        wsb, kk = wa_sb, k
                    elif k < 5:
                        wsb, kk = wb_sb, k - 3
                    else:
                        wsb, kk = wc_sb, k - 5
                    for h in range(N_HALF):
                        lhsT = wsb[:, kk * C_OUT + h * 128 : kk * C_OUT + h * 128 + 128]
                        nc.tensor.matmul(
                            pss[h][:], lhsT, rhs, start=(k == 0), stop=False
                        )

            def emit_finish(u):
                r0, nrows = units[u]
                n = nrows * W
                g8 = 0 if r0 < x8_r0[1] else 1
                off8 = (r0 - x8_r0[g8]) * W
                pss = live.pop(u)
                for h in range(N_HALF):
                    # the two fp8 taps, contracted together in one DoubleRow
                    # pass (PE packs 2 fp8 weights per cell, virtual K=256)
                    nc.tensor.matmul(
                        pss[h][:],
                        w8_sb[:, :, h * 128 : h * 128 + 128],
                        x8_groups[g8][:, :, off8 : off8 + n],
                        start=False,
                        stop=True,
                        perf_mode=mybir.MatmulPerfMode.DoubleRow,
                    )
                b_ap = wc_sb[:, 2 * C_OUT :].bitcast(F32)
                ot = opool.tile([128, N_HALF, n], F16, tag="ot", name="ot")
                # h0 on VectorE, h1 on ScalarE in parallel: the PSUM banks
                # free as fast as possible (the ring recycle gates tile t+3's
                # matmuls) and the halves land in one SBUF tile
                nc.vector.tensor_scalar(
                    out=ot[:, 0, :],
                    in0=pss[0][:],
                    scalar1=1.0 / WSCALE,
                    scalar2=b_ap[:, 0:1],
                    op0=mybir.AluOpType.mult,
                    op1=mybir.AluOpType.add,
                )
                nc.scalar.activation(
                    ot[:, 1, :],
                    pss[1][:],
                    mybir.ActivationFunctionType.Identity,
                    bias=b_ap[:, 1:2],
                    scale=1.0 / WSCALE,
                )
                # one combined-halves DMA per unit; alternate the trigger
                # between the two HWDGE engines so consecutive units' output
                # triggers (~600ns serial each) run in parallel at the tail
                eng = nc.sync if u % 2 == 0 else nc.scalar
                eng.dma_start(o_d[:, :, r0 : r0 + nrows, :], ot[:])

            for u in range(len(units)):
                emit_fp16(u)
                if u >= DR_DEFER:
                    emit_finish(u - DR_DEFER)
            for u in range(len(units) - DR_DEFER, len(units)):
                emit_finish(u)
    nc.compile()
    return nc


def kernel(x, weight, bias):
    global LAST_RESULT
    if "nc" not in _NC_CACHE:
        _NC_CACHE["nc"] = _build()
    nc = _NC_CACHE["nc"]

    x = np.ascontiguousarray(np.asarray(x, dtype=np.float32))
    weight = np.asarray(weight, dtype=np.float32)
    bias = np.asarray(bias, dtype=np.float32)

    E4 = ml_dtypes.float8_e4m3

    # fp16 taps 0..6 (transposed to lhsT layout, pre-scaled) + bias columns
    wT = weight.transpose(1, 2, 3, 0).reshape(C_IN, KH * KW, C_OUT)
    w16 = np.empty((C_IN, N_FP16_TAPS * C_OUT + 2 * N_HALF), dtype=np.float16)
    w16[:, : N_FP16_TAPS * C_OUT] = (wT[:, :N_FP16_TAPS, :] * WSCALE).reshape(
        C_IN, N_FP16_TAPS * C_OUT
    )
    # b[p, h] = bias[h*128 + p] in fp32, folded into the weight tensor as
    # raw bytes in fp16-typed columns (device bitcasts back to fp32)
    bh = np.ascontiguousarray(bias.reshape(N_HALF, 128).T.astype(np.float32))
    w16[:, N_FP16_TAPS * C_OUT :] = bh.view(np.float16)

    # fp8 pair weights: w8[c, s, o] = e4m3(WSCALE * weight[o, c, tap_s])
    w8 = np.empty((C_IN, 2, C_OUT), dtype=E4)
    for s, (ky, kx) in enumerate(FP8_TAPS):
        w8[:, s, :] = (wT[:, ky * KW + kx, :] * WSCALE).astype(E4)

    # zero-padded fp16 image; per-core slices carry their halo rows
    xp = np.zeros((C_IN, H + 2, WP), dtype=np.float16)
    xp[:, 1 : H + 1, 1 : W + 1] = x.astype(np.float16)

    # fp8 image, quantized once, then pre-shifted per tap slot and cropped:
    # x8[c, s, y*W + x] = e4m3(xpad[c, y+2, x+1+s])  (taps (2,1),(2,2))
    x8full = np.zeros((C_IN, H + 2, WP), dtype=E4)
    x8full[:, 1 : H + 1, 1 : W + 1] = x.astype(E4)

    in_maps = []
    for c in range(N_CORES):
        y0 = c * H_S
        x8c = np.empty((C_IN, 2, H_S, W), dtype=E4)
        for s in range(2):
            x8c[:, s, :, :] = x8full[:, y0 + 2 : y0 + 2 + H_S, 1 + s : 1 + s + W]
        in_maps.append(
            {
                "x": np.ascontiguousarray(xp[:, y0 : y0 + HP, :]),
                "x8": np.ascontiguousarray(x8c.reshape(C_IN, 2, H_S * W)),
                "w": w16,
                "w8": w8,
            }
        )

    kw = dict(TRACE_KW)
    if TRACE:
        kw.setdefault("trace", True)
        kw.setdefault("trace_cores", [0])
    res = bass_utils.run_bass_kernel_spmd(
        nc, in_maps, core_ids=list(range(N_CORES)), **kw
    )
    LAST_RESULT = res

    out = np.empty((C_OUT, H, W), dtype=np.float32)
    for c in range(N_CORES):
        # device layout [p, half, y, x] -> channel h*128+p
        arr = res.results[c]["out"].astype(np.float32)
        out[:, c * H_S : (c + 1) * H_S, :] = arr.transpose(1, 0, 2, 3).reshape(
            C_OUT, H_S, W
        )
    return out


# revision 22
# speedup vs baseline: 1.0285x; 1.0285x over previous
"""Conv2d(128->256, 3x3, pad=1) over a 256x256 image, sharded across 8 trn2 cores.

Strategy
--------
x: (C_in=128, H=256, W=256) f32, weight: (256, 128, 3, 3), bias: (256,1,1).
C_in == 128 maps exactly onto the SBUF partition (contraction) dim, so the
conv is 9 accumulated matmuls (one per kernel tap) per output tile:

    out[co, y, x] = sum_{ky,kx} W[ky,kx].T @ xpad[:, y+ky, x+kx]   + bias

Sharding: split H across the 8 cores (32 output rows each). Each core gets a
pre-padded slice xpad (128, 34, 258) with halo rows / zero borders prepared on
the host, so the device program is uniform SPMD. Per core, output tiles are
2 rows x 256 cols = 512 pixels (one fp32 PSUM bank); for each tile and each
128-channel output half we accumulate the taps, then VectorE applies
(psum * 1/512 + bias) on the PSUM->SBUF copy and the tile is DMA'd to DRAM
as fp16 (host upconverts; ~2^-11 rounding is far inside the error budget).

Mixed precision: 7 taps run as fp16 matmuls (full 16-bit PE rate, fp32 PSUM
accumulation). The last two taps (2,1),(2,2) run as ONE fp8e4m3 DoubleRow
matmul: the PE packs two fp8 weights per cell (virtual K=256), contracting
both taps in a single pass at ~1.1x the cost of one fp16 matmul instead of
two. Measured end-to-end rel err of this split on the reference distribution
is ~1.74e-2 (< the 2e-2 gate); pure fp8 would be ~3.4e-2 and fails.

All weights (fp16 and fp8) are pre-scaled by 512 on the host so the fp8
values land in e4m3's normal range (|w| <= 0.0295 would otherwise be ~50%
denormal, 20%+ quantization error); PSUM therefore holds 512*conv and the
bias copy fuses the 1/512 descale.

DMA: the two HWDGE queues (sync- and scalar-engine triggered) share the 16
DMA engines, so a concurrent bulk transfer steals wire bandwidth from the
critical path per-packet (and Tile's scheduler hoists dep-free dma_starts,
so a second queue cannot be held back). A single sync-engine queue therefore
carries ALL transfers in strict need-order. The fp8 (DoubleRow) matmuls of
tile t are deferred to after tile t+2's fp16 block so their data is never on
the critical path at all.
"""

import numpy as np
import ml_dtypes

import concourse.bass as bass
import concourse.tile as tile
from concourse import bacc, mybir
from concourse import bass_utils

N_CORES = 8
C_IN, C_OUT, KH, KW = 128, 256, 3, 3
H, W = 256, 256
H_S = H // N_CORES            # 32 output rows per core
HP, WP = H_S + 2, W + 2       # padded per-core input slice: 34 x 258
ROWS = 2                      # output rows per PSUM tile (N = ROWS*W = 512)
N_TILES = H_S // ROWS         # 16
N_HALF = C_OUT // 128         # 2 output-channel halves

F32 = mybir.dt.float32
F16 = mybir.dt.float16
F8 = mybir.dt.float8e4

# taps 0..6 = (0,0)..(2,0) in fp16; taps (2,1),(2,2) fused into one fp8
# DoubleRow matmul (see module docstring)
N_FP16_TAPS = 7
FP8_TAPS = [(2, 1), (2, 2)]
WSCALE = 512.0                # weight pre-scale; descaled in the bias copy

# x is split into row groups, each its own SBUF tile, so a group's matmuls
# can start as soon as its rows have landed (Tile deps are whole-tile). Each
# group covers GROUP_TILES[g] output tiles plus a 2-row halo overlap. The
# first group is 1 tile so the first matmuls start as early as possible.
# Few, large groups: every HWDGE trigger costs ~650ns of serial Sync-engine
# time and small transfers (small per-partition spans) get poor wire rates.
GROUP_TILES = [1, 3, 12]
assert sum(GROUP_TILES) == N_TILES
N_GROUPS = len(GROUP_TILES)

# fp8 pre-shifted x: [128, 2, H_S*W] per core, split into 2 groups (tiles).
X8_GROUP_TILES = [6, 10]
DR_DEFER = 2                  # run tile t's fp8 matmuls after tile t+2's fp16

# dep-free dummy matmuls issued at program start: they run while the input
# DMAs are in flight and lift the PE clock gate (HAM) out of its cold 1.2 GHz
# state before the real matmul stream begins. Sized to bridge engine-boot
# (~6.6us) to first-data (~10us): ~2.8us at the cold rate, with short
# N=128 warmups at the end for fine granularity. Keeping the PE busy until
# real data arrives also avoids an idle gap that would reset the HAM ramp.
WARMUP_512 = 9
WARMUP_128 = 0

# Set by test harness: TRACE=True makes the next kernel() call capture an
# NTFF profile; the BassKernelResults lands in LAST_RESULT.
TRACE = False
TRACE_KW = {}
LAST_RESULT = None

_NC_CACHE = {}


def _build():
    nc = bacc.Bacc(
        "TRN2",
        target_bir_lowering=False,
        debug=False,
        enable_asserts=False,
        num_devices=N_CORES,
    )
    x_d = nc.dram_tensor("x", [C_IN, HP, WP], F16, kind="ExternalInput").ap()
    x8_d = nc.dram_tensor("x8", [C_IN, 2, H_S * W], F8, kind="ExternalInput").ap()
    # fp16 taps 0..6 plus the fp32 bias folded in as 4 trailing fp16-typed
    # columns holding raw fp32 bytes (a separate bias tensor would be an
    # 8-byte-per-partition transfer — terrible packet size for the critical
    # DMA queue; and tensor_scalar requires an fp32 scalar2 operand)
    W_COLS = N_FP16_TAPS * C_OUT + 2 * N_HALF
    w_d = nc.dram_tensor("w", [C_IN, W_COLS], F16, kind="ExternalInput").ap()
    w8_d = nc.dram_tensor("w8", [C_IN, 2, C_OUT], F8, kind="ExternalInput").ap()
    # output laid out [p, half, y, x] (channel h*128+p at [p, h]) so ONE
    # DMA per unit moves both halves: each dma_start trigger costs ~600ns of
    # serial engine time, and the last units' triggers gate the kernel tail
    o_d = nc.dram_tensor(
        "out", [128, N_HALF, H_S, W], F16, kind="ExternalOutput"
    ).ap()

    with tile.TileContext(nc) as tc:
        with (
            tc.tile_pool(name="xin", bufs=1) as xpool,
            tc.tile_pool(name="x8in", bufs=1) as x8pool,
            tc.tile_pool(name="wts", bufs=1) as wpool,
            tc.tile_pool(name="acc", bufs=8, space="PSUM") as ppool,
            tc.tile_pool(name="outs", bufs=6) as opool,
        ):
            # PE warmup: dep-free. The scratch operand is a raw (statically
            # allocated) SBUF tensor that is never written — its garbage
            # contents stream through the PE and land in a scratch PSUM bank
            # nobody reads.
            warm_sb = nc.alloc_sbuf_tensor("warm_src", [128, ROWS * W], F16).ap()
            warm_ps = ppool.tile([128, ROWS * W], F32, tag="ps", name="ps")
            for _ in range(WARMUP_512):
                nc.tensor.matmul(warm_ps[:], warm_sb[:, :128], warm_sb[:])
            for _ in range(WARMUP_128):
                nc.tensor.matmul(warm_ps[:, :128], warm_sb[:, :128], warm_sb[:, :128])

            # three separate tiles: Tile dependencies are whole-tile, so
            # early taps must not wait for the later tap transfers
            wa_sb = wpool.tile([128, 3 * C_OUT], F16, tag="wa", name="wa")
            wb_sb = wpool.tile([128, 2 * C_OUT], F16, tag="wb", name="wb")
            wc_sb = wpool.tile([128, W_COLS - 5 * C_OUT], F16, tag="wc", name="wc")
            w8_sb = wpool.tile([128, 2, C_OUT], F8, tag="w8", name="w8")
            group_rows = [gt * ROWS + 2 for gt in GROUP_TILES]
            group_rows[0] = 3  # ky=2 rows live in the separate xg0b tile
            group_t0 = [sum(GROUP_TILES[:g]) for g in range(N_GROUPS)]
            x_groups = [
                xpool.tile([128, group_rows[g], WP], F16, tag=f"xg{g}", name=f"xg{g}")
                for g in range(N_GROUPS)
            ]
            xg0b = xpool.tile([128, 2, WP], F16, tag="xg0b", name="xg0b")
            x8_r0 = [0, X8_GROUP_TILES[0] * ROWS]
            x8_groups = [
                x8pool.tile(
                    [128, 2, gt * ROWS * W], F8, tag=f"x8g{g}", name=f"x8g{g}"
                )
                for g, gt in enumerate(X8_GROUP_TILES)
            ]

            # A single queue carries EVERYTHING in strict need-order: the
            # two HWDGE queues share the 16 DMA engines, so a "parallel"
            # second queue just steals wire bandwidth from the critical path
            # (and Tile's scheduler hoists dep-free dma_starts, so the
            # second queue cannot be held back). Need-order with the fp8
            # work deferred 2 tiles keeps every transfer ahead of its use.
            nc.sync.dma_start(x_groups[0][:], x_d[:, :3, :])
            nc.sync.dma_start(wa_sb[:], w_d[:, : 3 * C_OUT])
            nc.sync.dma_start(xg0b[:], x_d[:, 2:4, :])
            nc.sync.dma_start(wb_sb[:], w_d[:, 3 * C_OUT : 5 * C_OUT])
            nc.sync.dma_start(wc_sb[:], w_d[:, 5 * C_OUT :])
            r1 = group_t0[1] * ROWS
            nc.sync.dma_start(x_groups[1][:], x_d[:, r1 : r1 + group_rows[1], :])
            nc.sync.dma_start(w8_sb[:], w8_d[:])
            nc.sync.dma_start(
                x8_groups[0][:], x8_d[:, :, : X8_GROUP_TILES[0] * ROWS * W]
            )
            r2 = group_t0[2] * ROWS
            nc.sync.dma_start(x_groups[2][:], x_d[:, r2 : r2 + group_rows[2], :])
            nc.sync.dma_start(
                x8_groups[1][:], x8_d[:, :, X8_GROUP_TILES[0] * ROWS * W :]
            )

            def group_of_r0(r0):
                for g in reversed(range(N_GROUPS)):
                    if r0 >= group_t0[g] * ROWS:
                        return g
                raise AssertionError(r0)

            # processing units: 15 2-row tiles + two 1-row subtiles (the
            # split halves the bias-add + DMA latency off the final matmul)
            units = [(t * ROWS, ROWS) for t in range(N_TILES - 1)]
            units += [(H_S - 2, 1), (H_S - 1, 1)]
            live = {}

            def emit_fp16(u):
                r0, nrows = units[u]
                n = nrows * W
                g = group_of_r0(r0)
                yl = r0 - group_t0[g] * ROWS
                xg = x_groups[g]
                pss = [
                    ppool.tile([128, n], F32, tag="ps", name="ps")
                    for _ in range(N_HALF)
                ]
                live[u] = pss
                for k in range(N_FP16_TAPS):
                    ky, kx = divmod(k, KW)
                    if g == 0 and ky == 2:
                        rhs = xg0b[:, :nrows, kx : kx + W]
                    else:
                        rhs = xg[:, yl + ky : yl + ky + nrows, kx : kx + W]
                    if k < 3:
                        wsb, kk = wa_sb, k
                    elif k < 5:
                        wsb, kk = wb_sb, k - 3
                    else:
                        wsb, kk = wc_sb, k - 5
                    for h in range(N_HALF):
                        lhsT = wsb[:, kk * C_OUT + h * 128 : kk * C_OUT + h * 128 + 128]
                        nc.tensor.matmul(
                            pss[h][:], lhsT, rhs, start=(k == 0), stop=False
                        )

            def emit_finish(u):
                r0, nrows = units[u]
                n = nrows * W
                g8 = 0 if r0 < x8_r0[1] else 1
                off8 = (r0 - x8_r0[g8]) * W
                pss = live.pop(u)
                for h in range(N_HALF):
                    # the two fp8 taps, contracted together in one DoubleRow
                    # pass (PE packs 2 fp8 weights per cell, virtual K=256)
                    nc.tensor.matmul(
                        pss[h][:],
                        w8_sb[:, :, h * 128 : h * 128 + 128],
                        x8_groups[g8][:, :, off8 : off8 + n],
                        start=False,
                        stop=True,
                        perf_mode=mybir.MatmulPerfMode.DoubleRow,
                    )
                b_ap = wc_sb[:, 2 * C_OUT :].bitcast(F32)
                ot = opool.tile([128, N_HALF, n], F16, tag="ot", name="ot")
                # h0 on VectorE, h1 on ScalarE in parallel: the PSUM banks
                # free as fast as possible (the ring recycle gates tile t+3's
                # matmuls) and the halves land in one SBUF tile
                nc.vector.tensor_scalar(
                    out=ot[:, 0, :],
                    in0=pss[0][:],
                    scalar1=1.0 / WSCALE,
                    scalar2=b_ap[:, 0:1],
                    op0=mybir.AluOpType.mult,
                    op1=mybir.AluOpType.add,
                )
                nc.scalar.activation(
                    ot[:, 1, :],
                    pss[1][:],
                    mybir.ActivationFunctionType.Identity,
                    bias=b_ap[:, 1:2],
                    scale=1.0 / WSCALE,
                )
                # one combined-halves DMA per unit; alternate the trigger
                # between the two HWDGE engines so consecutive units' output
                # triggers (~600ns serial each) run in parallel at the tail
                eng = nc.sync if u % 2 == 0 else nc.scalar
                eng.dma_start(o_d[:, :, r0 : r0 + nrows, :], ot[:])

            for u in range(len(units)):
                emit_fp16(u)
                if u >= DR_DEFER:
                    emit_finish(u - DR_DEFER)
            for u in range(len(units) - DR_DEFER, len(units)):
                emit_finish(u)
    nc.compile()
    return nc


def kernel(x, weight, bias):
    global LAST_RESULT
    if "nc" not in _NC_CACHE:
        _NC_CACHE["nc"] = _build()
    nc = _NC_CACHE["nc"]

    x = np.ascontiguousarray(np.asarray(x, dtype=np.float32))
    weight = np.asarray(weight, dtype=np.float32)
    bias = np.asarray(bias, dtype=np.float32)

    E4 = ml_dtypes.float8_e4m3

    # fp16 taps 0..6 (transposed to lhsT layout, pre-scaled) + bias columns
    wT = weight.transpose(1, 2, 3, 0).reshape(C_IN, KH * KW, C_OUT)
    w16 = np.empty((C_IN, N_FP16_TAPS * C_OUT + 2 * N_HALF), dtype=np.float16)
    w16[:, : N_FP16_TAPS * C_OUT] = (wT[:, :N_FP16_TAPS, :] * WSCALE).reshape(
        C_IN, N_FP16_TAPS * C_OUT
    )
    # b[p, h] = bias[h*128 + p] in fp32, folded into the weight tensor as
    # raw bytes in fp16-typed columns (device bitcasts back to fp32)
    bh = np.ascontiguousarray(bias.reshape(N_HALF, 128).T.astype(np.float32))
    w16[:, N_FP16_TAPS * C_OUT :] = bh.view(np.float16)

    # fp8 pair weights: w8[c, s, o] = e4m3(WSCALE * weight[o, c, tap_s])
    w8 = np.empty((C_IN, 2, C_OUT), dtype=E4)
    for s, (ky, kx) in enumerate(FP8_TAPS):
        w8[:, s, :] = (wT[:, ky * KW + kx, :] * WSCALE).astype(E4)

    # zero-padded fp16 image; per-core slices carry their halo rows
    xp = np.zeros((C_IN, H + 2, WP), dtype=np.float16)
    xp[:, 1 : H + 1, 1 : W + 1] = x.astype(np.float16)

    # fp8 image, quantized once, then pre-shifted per tap slot and cropped:
    # x8[c, s, y*W + x] = e4m3(xpad[c, y+2, x+1+s])  (taps (2,1),(2,2))
    x8full = np.zeros((C_IN, H + 2, WP), dtype=E4)
    x8full[:, 1 : H + 1, 1 : W + 1] = x.astype(E4)

    in_maps = []
    for c in range(N_CORES):
        y0 = c * H_S
        x8c = np.empty((C_IN, 2, H_S, W), dtype=E4)
        for s in range(2):
            x8c[:, s, :, :] = x8full[:, y0 + 2 : y0 + 2 + H_S, 1 + s : 1 + s + W]
        in_maps.append(
            {
                "x": np.ascontiguousarray(xp[:, y0 : y0 + HP, :]),
                "x8": np.ascontiguousarray(x8c.reshape(C_IN, 2, H_S * W)),
                "w": w16,
                "w8": w8,
            }
        )

    kw = dict(TRACE_KW)
    if TRACE:
        kw.setdefault("trace", True)
        kw.setdefault("trace_cores", [0])
    res = bass_utils.run_bass_kernel_spmd(
        nc, in_maps, core_ids=list(range(N_CORES)), **kw
    )
    LAST_RESULT = res

    out = np.empty((C_OUT, H, W), dtype=np.float32)
    for c in range(N_CORES):
        # device layout [p, half, y, x] -> channel h*128+p
        arr = res.results[c]["out"].astype(np.float32)
        out[:, c * H_S : (c + 1) * H_S, :] = arr.transpose(1, 0, 2, 3).reshape(
            C_OUT, H_S, W
        )
    return out
